# revision 44
# baseline (speedup 1.0000x reference)
"""Trainium2 Bass kernel for nn_Detector (GNN message passing).

Math: the reference's per-iteration edge aggregation
    agg = segment_sum((h[src] + ef_w[ef]) * valid, by=ed)[:N] / cnt
is linear in h and ef_w, so it factors through two tiny count histograms
built in ONE pass over the edge index arrays:
    C[d, s] = #valid edges s->d          (32x32)
    F[d, t] = #valid edges into d with feature t   (32x6)
    agg = (C @ h + F @ ef_w) / cnt,   cnt = max(rowsum(C), 1)
Out-of-range (padded) edges produce all-zero one-hot rows and drop out
automatically, matching the reference's valid-mask semantics.

Distribution: edges are sharded across 8 cores; each core builds partial
C|F [32,38] via one-hot matmuls (contraction over 128-edge chunks on the
PE), partials are AllGather'ed and summed, then every core runs the
identical 5-iteration GRU + head on [32,128] tiles; core 0's scalar is
returned.

v2 layout/scheduling changes vs the first working version:
  - edge phase emitted FIRST (engine queues are in-order; weight prep
    used to stall the edge one-hots ~20us behind small-parameter DMAs)
  - one-hots built value-major with 38 tensor_scalar is_equal ops (DVE
    4x mode, ~170-270ns each) instead of one broadcast tensor_tensor
    (1x mode, 13.2us)
  - a dummy warm-up AllReduce at kernel start absorbs inter-core skew
    so the real AllGather (issued right after the partials) doesn't eat
    a ~16us rendezvous wait; all GRU weight prep + h0 build happens in
    the collective's shadow
  - GRU gate matmuls run in bf16 (PE is pinned at 1.2GHz here and fp32
    moving operands cost 2 passes), LN stats via bn_stats/bn_aggr, and
    the new h is produced in both layouts by parallel engine branches
    (PE+ACT for hT, DVE for h) instead of a serial transpose-back.
"""

import ml_dtypes
import numpy as np

import concourse.bass as bass
import concourse.mybir as mybir
import concourse.tile as tile
from concourse import library_config
from concourse.tile import add_dep_helper
from concourse.bass_utils import run_bass_kernel_spmd

dt = mybir.dt
AF = mybir.ActivationFunctionType
ALU = mybir.AluOpType

NCORES = 8
E_FULL = 400000
W = 392                    # edge columns per partition row
EPC = 128 * W              # 50176 padded edges per core
E_PAD = NCORES * EPC       # 401408
NGRP = W // 4              # 98 matmul groups of 4 chunks (512 edges)
DIM = 128
N = 32
EPS = 1e-5
RSQRT_MAGIC = 0x5F3759DF   # rsqrt bit-hack seed
MAX_WAITS = 1              # this walrus rejects >1 sync wait per instruction


def _split_excess_waits(nc):
    """Split instructions carrying more than MAX_WAITS sync-wait conditions
    into preceding same-engine NOPs (walrus codegen limit)."""
    for blk in nc.main_func.blocks:
        insts = blk.instructions
        i = 0
        while i < len(insts):
            inst = insts[i]
            si = inst.sync_info
            if si is not None and len(si.on_wait) > MAX_WAITS:
                waits = list(si.on_wait)
                keep = waits[-MAX_WAITS:]
                rest = waits[:-MAX_WAITS]
                new_nops = []
                while rest:
                    chunk, rest = rest[:MAX_WAITS], rest[MAX_WAITS:]
                    nop = mybir.InstNoOp(
                        name=f"waitsplit-{nc.next_id()}", ins=[], outs=[])
                    nop.engine = inst.engine
                    nop.sync_info = mybir.SyncInfo(on_wait=chunk, on_update=[])
                    nc.register_instruction(nop, overwrite=True)
                    new_nops.append(nop)
                inst.sync_info = mybir.SyncInfo(
                    on_wait=keep, on_update=list(si.on_update))
                for j, nop in enumerate(new_nops):
                    insts.insert(i + j, nop)
                i += len(new_nops)
            i += 1

f32 = dt.float32
bf16 = dt.bfloat16
i16 = dt.int16
i32 = dt.int32


def _splice_sem_wait(inst, sem, value):
    """Append a cross-core semaphore wait to an already-scheduled marker
    instruction (the Tile scheduler's single-core sim cannot satisfy these
    waits, so they are added after scheduling)."""
    si = inst.sync_info
    old_waits = list(si.on_wait) if si is not None else []
    old_upds = list(si.on_update) if si is not None else []
    w = mybir.SyncWait(sync_type="semaphore", id=sem.num, ant_name=sem.name,
                       wait_mode="sem-ge-imm", wait_value=value, wait_reg=None)
    inst.sync_info = mybir.SyncInfo(on_wait=old_waits + [w],
                                    on_update=old_upds)


def _rsqrt_newton(nc, vp, u, tag_prefix, iters=2):
    """1/sqrt(u) for u [P,1] fp32 in SBUF via the rsqrt bit-hack seed +
    Newton iterations using only mult/add (this walrus cannot encode
    AP-scalar divide; ACT Sqrt lives in a different table -> ~1.3us
    switch per use). Returns [P,1] inv_sigma AP."""
    P = u.shape[0]
    y = vp.tile([P, 1], f32, name=f"{tag_prefix}_y", tag=f"{tag_prefix}_y")
    a = vp.tile([P, 1], f32, name=f"{tag_prefix}_a", tag=f"{tag_prefix}_a")
    # y0 bits = MAGIC - (u_bits >> 1), via c - x = (~x) + (c + 1)
    # (bitwise and arith ALU ops cannot share one instruction)
    nc.vector.tensor_scalar(
        y.bitcast(i32), u.bitcast(i32), 1, None, ALU.logical_shift_right)
    nc.vector.tensor_scalar(
        y.bitcast(i32), y.bitcast(i32), -1, None, ALU.bitwise_xor)
    nc.vector.tensor_scalar(
        y.bitcast(i32), y.bitcast(i32), RSQRT_MAGIC + 1, None, ALU.add)
    for _ in range(iters):
        nc.vector.tensor_mul(a, y, y)                             # y^2
        nc.vector.tensor_mul(a, a, u)                             # u*y^2
        nc.vector.tensor_scalar(a, a, -0.5, 1.5, ALU.mult, ALU.add)
        nc.vector.tensor_mul(y, y, a)                             # Newton
    return y


def build_program():
    # this walrus snapshot cannot encode the Pool RANGE_CLEAR InstISA that
    # TileContext's exit emits via clear_and_free_semaphores; skip the
    # sem-clear ISA (keep dma_reset + bookkeeping).  The NEFF is executed
    # freshly per load, so end-of-kernel sem hygiene is not load-bearing
    # here (verified by back-to-back runs in test.py).
    _orig_clear = bass.Bass.clear_and_free_semaphores

    def _clear_no_isa(self, sems):
        if not sems:
            return
        sem_nums = [
            s.num if isinstance(s, bass.SemaphoreHandle) else s for s in sems
        ]
        from concourse.bass import compact_to_ranges
        for sem_range in compact_to_ranges(sem_nums):
            self.gpsimd.dma_reset(sem_range)
        self._state.prepend_free_semaphores(sem_nums)
        for poison_set in self._tile_sem_poison_stack:
            poison_set.update(sem_nums)

    bass.Bass.clear_and_free_semaphores = _clear_no_isa
    try:
        return _build_program_inner()
    finally:
        bass.Bass.clear_and_free_semaphores = _orig_clear


def _build_program_inner():
    nc = bass.Bass(trn_type="TRN2")

    # ---- DRAM I/O ---------------------------------------------------------
    es_d = nc.dram_tensor("es", [128, 4 * W], i16, kind="ExternalInput")
    ed_d = nc.dram_tensor("ed", [128, 4 * W], i16, kind="ExternalInput")
    ef_d = nc.dram_tensor("ef", [128, 4 * W], i16, kind="ExternalInput")
    nt_d = nc.dram_tensor("nt", [32, 4], i16, kind="ExternalInput")
    tr_d = nc.dram_tensor("tr", [32, 4], i16, kind="ExternalInput")
    ne_w_d = nc.dram_tensor("ne_w", [20, DIM], f32, kind="ExternalInput")
    te_w_d = nc.dram_tensor("te_w", [6, DIM], f32, kind="ExternalInput")
    ef_w_d = nc.dram_tensor("ef_w", [6, DIM], f32, kind="ExternalInput")
    w_ih_d = nc.dram_tensor("w_ih", [3 * DIM, DIM], f32, kind="ExternalInput")
    w_hh_d = nc.dram_tensor("w_hh", [3 * DIM, DIM], f32, kind="ExternalInput")
    b_ih_d = nc.dram_tensor("b_ih", [1, 3 * DIM], f32, kind="ExternalInput")
    b_hh_d = nc.dram_tensor("b_hh", [1, 3 * DIM], f32, kind="ExternalInput")
    ln_g_d = nc.dram_tensor("ln_g", [DIM, 1], f32, kind="ExternalInput")
    ln_b_d = nc.dram_tensor("ln_b", [DIM, 1], f32, kind="ExternalInput")
    fc1_w_d = nc.dram_tensor("fc1_w", [DIM, 2 * DIM], f32, kind="ExternalInput")
    fc1_b_d = nc.dram_tensor("fc1_b", [DIM, 1], f32, kind="ExternalInput")
    ln2_g_d = nc.dram_tensor("ln2_g", [DIM, 1], f32, kind="ExternalInput")
    ln2_b_d = nc.dram_tensor("ln2_b", [DIM, 1], f32, kind="ExternalInput")
    fc2_w_d = nc.dram_tensor("fc2_w", [1, DIM], f32, kind="ExternalInput")
    fc2_b_d = nc.dram_tensor("fc2_b", [1, 1], f32, kind="ExternalInput")
    ident_d = nc.dram_tensor("ident128", [128, 128], f32, kind="ExternalInput")
    ones_r_d = nc.dram_tensor("ones_row", [1, 128], f32, kind="ExternalInput")
    ones_c_d = nc.dram_tensor("ones_col", [128, 1], f32, kind="ExternalInput")
    iota_m_d = nc.dram_tensor("iota_mat", [32, 32], f32, kind="ExternalInput")
    iota_b_d = nc.dram_tensor("iota_row_bf", [128, 32], dt.bfloat16,
                              kind="ExternalInput")
    out_d = nc.dram_tensor("out", [1, 1], f32, kind="ExternalOutput")

    # collective bounce buffers (internal DRAM)
    ag_in = nc.dram_tensor("ag_in", [32, 38], f32)
    ag_out = nc.dram_tensor("ag_out", [32 * NCORES, 38], f32, addr_space="Shared")

    with tile.TileContext(nc) as tc:
        with (
            tc.tile_pool(name="cst", bufs=1) as cp,      # persistent SBUF
            tc.tile_pool(name="var", bufs=2) as vp,      # loop temporaries
            tc.tile_pool(name="ps", bufs=1, space="PSUM") as pp,
        ):
            # ========== edge DMAs first (biggest transfer, queues idle) ====
            raw = cp.tile([128, 3 * 4 * W], i16, name="raw")
            nc.sync.dma_start(raw[:, 0:4 * W], es_d[:, :])
            nc.sync.dma_start(raw[:, 8 * W:12 * W], ef_d[:, :])
            nc.sync.dma_start(raw[:, 4 * W:8 * W], ed_d[:, :])



            # warm the ACT table (sigmoid set) early, off the critical path
            ones_col = cp.tile([128, 1], f32, name="ones_col_sb")
            nc.sync.dma_start(ones_col, ones_c_d[:, :])
            act_warm = cp.tile([1, 1], f32, name="act_warm")
            nc.scalar.activation(act_warm, ones_col[0:1, 0:1], AF.Sigmoid)

            # remaining parameter DMAs (issue now; engines consume later)
            ident = cp.tile([128, 128], f32, name="ident")
            nc.sync.dma_start(ident, ident_d[:, :])
            ones_row = cp.tile([1, 128], f32, name="ones_row_sb")
            nc.sync.dma_start(ones_row, ones_r_d[:, :])
            iota_mat = cp.tile([32, 32], f32, name="iota_mat_sb")
            nc.sync.dma_start(iota_mat, iota_m_d[:, :])
            iota_bf = cp.tile([128, 32], bf16, name="iota_bf_sb")
            nc.sync.dma_start(iota_bf, iota_b_d[:, :])
            ne_w = cp.tile([20, DIM], f32, name="ne_w_sb")
            nc.sync.dma_start(ne_w, ne_w_d[:, :])
            te_w = cp.tile([6, DIM], f32, name="te_w_sb")
            nc.sync.dma_start(te_w, te_w_d[:, :])
            ef_w = cp.tile([6, DIM], f32, name="ef_w_sb")
            nc.sync.dma_start(ef_w, ef_w_d[:, :])
            b_ih = cp.tile([1, 384], f32, name="b_ih_sb")
            nc.sync.dma_start(b_ih, b_ih_d[:, :])
            b_hh = cp.tile([1, 384], f32, name="b_hh_sb")
            nc.sync.dma_start(b_hh, b_hh_d[:, :])
            ln_g = cp.tile([128, 1], f32, name="ln_g_sb")
            nc.sync.dma_start(ln_g, ln_g_d[:, :])
            ln_b = cp.tile([128, 1], f32, name="ln_b_sb")
            nc.sync.dma_start(ln_b, ln_b_d[:, :])
            # same 512B, viewed as a row for the ln scale/bias broadcasts
            ln_g_row = cp.tile([1, 128], f32, name="ln_g_row")
            nc.sync.dma_start(ln_g_row, ln_g_d.rearrange("d a -> a d"))
            ln_b_row = cp.tile([1, 128], f32, name="ln_b_row")
            nc.sync.dma_start(ln_b_row, ln_b_d.rearrange("d a -> a d"))
            fc1_b = cp.tile([128, 1], f32, name="fc1_b_sb")
            nc.sync.dma_start(fc1_b, fc1_b_d[:, :])
            ln2_g = cp.tile([128, 1], f32, name="ln2_g_sb")
            nc.sync.dma_start(ln2_g, ln2_g_d[:, :])
            ln2_b = cp.tile([128, 1], f32, name="ln2_b_sb")
            nc.sync.dma_start(ln2_b, ln2_b_d[:, :])
            fc2_col = cp.tile([128, 1], f32, name="fc2_col")
            nc.sync.dma_start(fc2_col, fc2_w_d.rearrange("a d -> d a"))
            fc2_b = cp.tile([1, 1], f32, name="fc2_b_sb")
            nc.sync.dma_start(fc2_b, fc2_b_d[:, :])
            fc1_w = cp.tile([128, 256], f32, name="fc1_w_sb")
            nc.sync.dma_start(fc1_w, fc1_w_d[:, :])
            w_ih_raw = cp.tile([128, 3 * 128], f32, name="w_ih_raw")
            w_hh_raw = cp.tile([128, 3 * 128], f32, name="w_hh_raw")
            for g in range(3):
                nc.sync.dma_start(
                    w_ih_raw[:, 128 * g:128 * (g + 1)],
                    w_ih_d[128 * g:128 * (g + 1), :])
                nc.sync.dma_start(
                    w_hh_raw[:, 128 * g:128 * (g + 1)],
                    w_hh_d[128 * g:128 * (g + 1), :])
            nt_c16 = cp.tile([32, 1], i16, name="nt_c16")
            tr_c16 = cp.tile([32, 1], i16, name="tr_c16")
            nc.sync.dma_start(nt_c16, nt_d[:, 0:1])
            nc.sync.dma_start(tr_c16, tr_d[:, 0:1])

            # ========== edge phase: one-hots (DVE, 4x mode) ================
            # compact int64-low-halves (stride 4 int16) -> unit-stride bf16;
            # one cast per array so es one-hots start before ed/ef DMAs land
            sd = cp.tile([128, 3 * W], bf16, name="sd")
            raw_v = raw.rearrange("p (c w f) -> p c w f", c=3, f=4)
            nc.vector.tensor_copy(sd[:, 0:W], raw_v[:, 0, :, 0])
            nc.vector.tensor_copy(sd[:, 2 * W:3 * W], raw_v[:, 2, :, 0])
            nc.vector.tensor_copy(sd[:, W:2 * W], raw_v[:, 1, :, 0])

            # one-hot planes:
            #   ohsf plane v (width W, value-major): v<32 -> [es==v],
            #   v=32+u -> [ef==u]; es and ef share one tile so a single
            #   2D moving AP [128, 4, 38] covers both per matmul group.
            #   ohd chunk-major [p, c*32+v] (stationary MUST be a single
            #   free dim for walrus -> contiguous 128-col group slices)
            # es/ef builds are tensor_scalar is_equal at DVE 4x mode; the
            # ed build is a broadcast tensor_tensor (1x mode) split into
            # pieces so the histogram matmuls pipeline behind it (last
            # piece on gpsimd as an engine-offload probe).
            ohsf = cp.tile([128, 38 * W], bf16, name="ohsf")
            ohd = cp.tile([128, W * 32], bf16, name="ohd")
            for v in range(32):
                nc.vector.tensor_scalar(
                    ohsf[:, v * W:(v + 1) * W], sd[:, 0:W],
                    float(v), None, ALU.is_equal)
            for u in range(6):
                nc.vector.tensor_scalar(
                    ohsf[:, (32 + u) * W:(33 + u) * W], sd[:, 2 * W:3 * W],
                    float(u), None, ALU.is_equal)
            ohd_r = ohd.rearrange("p (c v) -> p c v", v=32)
            PIECES = [(0, 96), (96, 192), (192, 288), (288, 392)]
            for c0, c1 in PIECES:
                nc.vector.tensor_tensor(
                    ohd_r[:, c0:c1, :],
                    sd[:, W + c0:W + c1].unsqueeze(2)
                    .broadcast_to([128, c1 - c0, 32]),
                    iota_bf.unsqueeze(1).broadcast_to([128, c1 - c0, 32]),
                    ALU.is_equal)

            # ========== histogram matmuls ==================================
            # psum[(c,d),(c',u)] += ED_c^T [ES|EF]_c' over groups of 4
            # chunks; only the 4 diagonal c==c' [32,38] blocks matter.
            hist = pp.tile([128, 152], f32, name="hist", tag="psA")
            ohsf_r = ohsf.rearrange("p (v c) -> p c v", v=38)
            for g in range(NGRP):
                nc.tensor.matmul(hist, ohd[:, 128 * g:128 * (g + 1)],
                                 ohsf_r[:, 4 * g:4 * g + 4, :],
                                 start=(g == 0), stop=(g == NGRP - 1))

            # extract + sum the 4 diagonal [32,38] blocks on the PE via
            # identity-selector matmuls (no cross-partition DMA latency):
            # pk = sum_j I[:,32j:32j+32]^T @ hs[:, 38j:38j+38]
            hs = cp.tile([128, 152], f32, name="hs")
            nc.scalar.copy(hs, hist)
            pk_ps = pp.tile([32, 38], f32, name="pk_ps", tag="psB")
            for j in range(4):
                nc.tensor.matmul(pk_ps, ident[:, 32 * j:32 * (j + 1)],
                                 hs[:, 38 * j:38 * (j + 1)],
                                 start=(j == 0), stop=(j == 3))
            pk = cp.tile([32, 38], f32, name="pk")
            nc.vector.tensor_copy(pk, pk_ps)

            # ========== AllGather partials (issued ASAP) ===================
            nc.sync.dma_start(ag_in.ap(), pk)
            nc.gpsimd.collective_compute(
                "AllGather", ALU.bypass,
                ins=[ag_in.ap().opt()], outs=[ag_out.ap().opt()],
                replica_groups=[list(range(NCORES))])

            # ========== weight prep + h0 (runs in the collective shadow) ===
            # GRU weights, transposed to [dim_in(K)=128, gate] bf16 layout
            w_ihT = cp.tile([128, 384], bf16, name="w_ihT")
            w_hhT = cp.tile([128, 384], bf16, name="w_hhT")
            for gsrc, gdst in ((w_ih_raw, w_ihT), (w_hh_raw, w_hhT)):
                for g in range(3):
                    wT_ps = pp.tile([128, 128], f32, name="wT_ps", tag="psB")
                    nc.tensor.transpose(
                        wT_ps, gsrc[:, 128 * g:128 * (g + 1)], ident)
                    nc.vector.tensor_copy(gdst[:, 128 * g:128 * (g + 1)], wT_ps)
            fc1T_a = cp.tile([128, 128], f32, name="fc1T_a")
            fc1T_b = cp.tile([128, 128], f32, name="fc1T_b")
            for g, gdst in enumerate((fc1T_a, fc1T_b)):
                wT_ps = pp.tile([128, 128], f32, name="wT_ps2", tag="psB")
                nc.tensor.transpose(wT_ps, fc1_w[:, 128 * g:128 * (g + 1)], ident)
                nc.scalar.copy(gdst, wT_ps)

            bsum = cp.tile([1, 384], f32, name="bsum")
            nc.vector.tensor_add(bsum, b_ih, b_hh)
            bsum_bf = cp.tile([1, 384], bf16, name="bsum_bf")
            nc.vector.tensor_copy(bsum_bf, bsum)
            b_ih_bf = cp.tile([1, 384], bf16, name="b_ih_bf")
            nc.vector.tensor_copy(b_ih_bf, b_ih)
            b_hh_bf = cp.tile([1, 384], bf16, name="b_hh_bf")
            nc.vector.tensor_copy(b_hh_bf, b_hh)
            ones_row_bf = cp.tile([1, 128], bf16, name="ones_row_bf")
            nc.vector.tensor_copy(ones_row_bf, ones_row)
            ef_w_bf = cp.tile([6, DIM], bf16, name="ef_w_bf")
            nc.vector.tensor_copy(ef_w_bf, ef_w)

            # ln gamma/beta replicated to [32,128] for the node-major affine
            lnrep_ps = pp.tile([32, 256], f32, name="lnrep_ps", tag="psC")
            nc.tensor.matmul(lnrep_ps[:, 0:128], ones_row[0:1, 0:32], ln_g_row,
                             start=True, stop=True)
            nc.tensor.matmul(lnrep_ps[:, 128:256], ones_row[0:1, 0:32], ln_b_row,
                             start=True, stop=True)
            g_rep = cp.tile([32, 128], f32, name="g_rep")
            b_rep = cp.tile([32, 128], f32, name="b_rep")
            nc.vector.tensor_copy(g_rep, lnrep_ps[:, 0:128])
            nc.vector.tensor_copy(b_rep, lnrep_ps[:, 128:256])

            # h0 = ne_w[nt] + te_w[tr]
            nt_col = cp.tile([32, 1], f32, name="nt_col")
            tr_col = cp.tile([32, 1], f32, name="tr_col")
            nc.vector.tensor_copy(nt_col, nt_c16)
            nc.vector.tensor_copy(tr_col, tr_c16)
            nt_oh = cp.tile([32, 32], f32, name="nt_oh")
            tr_oh = cp.tile([32, 32], f32, name="tr_oh")
            nc.vector.tensor_scalar(nt_oh, iota_mat, nt_col, None, ALU.is_equal)
            nc.vector.tensor_scalar(tr_oh, iota_mat, tr_col, None, ALU.is_equal)
            ntT = cp.tile([32, 32], f32, name="ntT")
            trT = cp.tile([32, 32], f32, name="trT")
            nc.vector.transpose(ntT, nt_oh)
            nc.vector.transpose(trT, tr_oh)
            h0_ps = pp.tile([32, 128], f32, name="h0_ps", tag="psD")
            nc.tensor.matmul(h0_ps, ntT[0:20, :], ne_w, start=True, stop=False)
            nc.tensor.matmul(h0_ps, trT[0:6, :], te_w, start=False, stop=True)
            h_sb = vp.tile([32, 128], f32, name="h_sb", tag="h_sb")
            nc.vector.tensor_copy(h_sb, h0_ps)
            hT_ps0 = pp.tile([128, 32], f32, name="hT_ps0", tag="psE")
            nc.tensor.transpose(hT_ps0, h_sb, ident[0:32, 0:32])
            hT_bf = vp.tile([128, 32], bf16, name="hT_bf", tag="hT_bf")
            hT0_cast = nc.vector.tensor_copy(hT_bf, hT_ps0)
            hT_f32 = None

            # ========== gathered partials -> M1T, FnT ======================
            g8 = cp.tile([32, 8 * 38], f32, name="g8")
            nc.sync.dma_start(
                g8.rearrange("p (i u) -> p i u", i=8),
                ag_out.ap().rearrange("(i d) u -> d i u", d=32))
            a4 = cp.tile([32, 152], f32, name="a4")
            nc.vector.tensor_add(a4, g8[:, 0:152], g8[:, 152:304])
            a2 = cp.tile([32, 76], f32, name="a2")
            nc.vector.tensor_add(a2, a4[:, 0:76], a4[:, 76:152])
            cf = cp.tile([32, 38], f32, name="cf")
            nc.vector.tensor_add(cf, a2[:, 0:38], a2[:, 38:76])

            cnt = cp.tile([32, 1], f32, name="cnt")
            nc.vector.reduce_sum(cnt, cf[:, 0:32], axis=mybir.AxisListType.X)
            nc.vector.tensor_scalar(cnt, cnt, 1.0, None, ALU.max)
            inv = cp.tile([32, 1], f32, name="inv")
            nc.vector.reciprocal(inv, cnt)
            m1 = cp.tile([32, 32], f32, name="m1")
            nc.vector.tensor_scalar(m1, cf[:, 0:32], inv, None, ALU.mult)
            m1T = cp.tile([32, 32], f32, name="m1T")
            nc.vector.transpose(m1T, m1)
            fn_pad = cp.tile([32, 32], f32, name="fn_pad")
            nc.vector.memset(fn_pad, 0.0)
            nc.vector.tensor_scalar(
                fn_pad[:, 0:6], cf[:, 32:38], inv, None, ALU.mult)
            fnT = cp.tile([32, 32], f32, name="fnT")
            nc.vector.transpose(fnT, fn_pad)

            # ========== 5 GRU iterations ===================================
            for it in range(5):
                # constant-input matmuls lead each PSUM accumulation group
                # (they only wait on the bank's WAR release, so they issue
                # long before aggT/h are ready and leave the critical path)
                aggT_ps = pp.tile([128, 32], f32, name="aggT_ps", tag="psA")
                nc.tensor.matmul(aggT_ps, ef_w, fnT[0:6, :],
                                 start=True, stop=False)
                nc.tensor.matmul(aggT_ps, h_sb, m1T, start=False, stop=True)
                aggT_bf = vp.tile([128, 32], bf16, name="aggT_bf", tag="aggT")
                nc.vector.tensor_copy(aggT_bf, aggT_ps)

                g_rz = pp.tile([32, 256], f32, name="g_rz", tag="psB")
                nc.tensor.matmul(g_rz, ones_row_bf[0:1, 0:32],
                                 bsum_bf[:, 0:256], start=True, stop=False)
                nc.tensor.matmul(g_rz, aggT_bf, w_ihT[:, 0:256],
                                 start=False, stop=False)
                nc.tensor.matmul(g_rz, hT_bf, w_hhT[:, 0:256],
                                 start=False, stop=True)
                hn_ps = pp.tile([32, 128], f32, name="hn_ps", tag="psC")
                nc.tensor.matmul(hn_ps, ones_row_bf[0:1, 0:32],
                                 b_hh_bf[:, 256:384], start=True, stop=False)
                nc.tensor.matmul(hn_ps, hT_bf, w_hhT[:, 256:384],
                                 start=False, stop=True)
                in_ps = pp.tile([32, 128], f32, name="in_ps", tag="psD")
                nc.tensor.matmul(in_ps, ones_row_bf[0:1, 0:32],
                                 b_ih_bf[:, 256:384], start=True, stop=False)
                nc.tensor.matmul(in_ps, aggT_bf, w_ihT[:, 256:384],
                                 start=False, stop=True)

                # split sigmoid so the r-half lands ~200ns sooner for t1
                rz = vp.tile([32, 256], f32, name="rz", tag="rz")
                nc.scalar.activation(rz[:, 0:128], g_rz[:, 0:128], AF.Sigmoid)
                nc.scalar.activation(rz[:, 128:256], g_rz[:, 128:256],
                                     AF.Sigmoid)
                t1 = vp.tile([32, 128], f32, name="t1", tag="t1")
                nc.vector.tensor_tensor(t1, rz[:, 0:128], hn_ps, ALU.mult)
                t2 = vp.tile([32, 128], f32, name="t2", tag="t2")
                nc.vector.tensor_tensor(t2, t1, in_ps, ALU.add)
                n_sb = vp.tile([32, 128], f32, name="n_sb", tag="n_sb")
                nc.scalar.activation(n_sb, t2, AF.Tanh)

                d1 = vp.tile([32, 128], f32, name="d1", tag="d1")
                nc.vector.tensor_sub(d1, h_sb, n_sb)
                t3 = vp.tile([32, 128], f32, name="t3", tag="t3")
                nc.vector.tensor_tensor(t3, rz[:, 128:256], d1, ALU.mult)
                x_sb = vp.tile([32, 128], f32, name="x_sb", tag="x_sb")
                nc.vector.tensor_add(x_sb, t3, n_sb)

                bn6 = vp.tile([32, 6], f32, name="bn6", tag="bn6")
                nc.vector.bn_stats(bn6, x_sb)
                mv2 = vp.tile([32, 2], f32, name="mv2", tag="mv2")
                nc.vector.bn_aggr(mv2, bn6)
                uv = vp.tile([32, 1], f32, name="uv", tag="uv")
                nc.vector.tensor_scalar(uv, mv2[:, 1:2], EPS, None, ALU.add)
                isg = _rsqrt_newton(nc, vp, uv, "it", iters=2 if it == 4 else 1)

                xn = vp.tile([32, 128], f32, name="xn", tag="xn")
                nc.vector.tensor_scalar(xn, x_sb, mv2[:, 0:1], isg,
                                        ALU.subtract, ALU.mult)
                # branch A (PE+ACT): hT with affine applied per-partition;
                # only the last iteration needs an f32 copy (head max-pool)
                xnT_ps = pp.tile([128, 32], f32, name="xnT_ps", tag="psE")
                nc.tensor.transpose(xnT_ps, xn, ident[0:32, 0:32])
                if it == 4:
                    hT_f32 = vp.tile([128, 32], f32, name="hT_f32",
                                     tag="hT_f32")
                    nc.scalar.activation(hT_f32, xnT_ps, AF.Identity,
                                         bias=ln_b, scale=ln_g)
                    hT_bf = vp.tile([128, 32], bf16, name="hT_bf", tag="hT_bf")
                    nc.vector.tensor_copy(hT_bf, hT_f32)
                else:
                    hT_bf = vp.tile([128, 32], bf16, name="hT_bf", tag="hT_bf")
                    nc.scalar.activation(hT_bf, xnT_ps, AF.Identity,
                                         bias=ln_b, scale=ln_g)
                # branch B (DVE): node-major h via replicated gamma/beta
                hg = vp.tile([32, 128], f32, name="hg", tag="hg")
                nc.vector.tensor_tensor(hg, xn, g_rep, ALU.mult)
                h_sb = vp.tile([32, 128], f32, name="h_sb", tag="h_sb")
                nc.vector.tensor_tensor(h_sb, hg, b_rep, ALU.add)

            # ========== head: pool + fc1 + LN2 + relu + fc2 ================
            mean_ps = pp.tile([128, 1], f32, name="mean_ps", tag="psE")
            nc.tensor.matmul(mean_ps, h_sb, ones_col[0:32, 0:1],
                             start=True, stop=True)
            mean_sb = cp.tile([128, 1], f32, name="mean_sb")
            nc.scalar.activation(mean_sb, mean_ps, AF.Identity, scale=1.0 / 32)
            max_sb = cp.tile([128, 1], f32, name="max_sb")
            nc.vector.reduce_max(max_sb, hT_f32, axis=mybir.AxisListType.X)

            x1_ps = pp.tile([128, 1], f32, name="x1_ps", tag="psF")
            nc.tensor.matmul(x1_ps, fc1T_a, mean_sb, start=True, stop=False)
            nc.tensor.matmul(x1_ps, fc1T_b, max_sb, start=False, stop=True)
            st_in = cp.tile([128, 2], f32, name="st_in")
            nc.vector.tensor_add(st_in[:, 0:1], x1_ps, fc1_b)
            nc.scalar.activation(st_in[:, 1:2], st_in[:, 0:1], AF.Square)
            st_ps = pp.tile([1, 2], f32, name="st_ps", tag="psC")
            nc.tensor.matmul(st_ps, ones_col, st_in, start=True, stop=True)

            m2 = cp.tile([1, 1], f32, name="m2")
            nc.vector.tensor_scalar(m2, st_ps[0:1, 0:1], 1.0 / 128, None,
                                    ALU.mult)
            a2v = cp.tile([1, 1], f32, name="a2v")
            nc.vector.tensor_scalar(a2v, st_ps[0:1, 1:2], 1.0 / 128, EPS,
                                    ALU.mult, ALU.add)
            b2v = cp.tile([1, 1], f32, name="b2v")
            nc.vector.tensor_scalar(b2v, m2, m2, None, ALU.mult)
            u2 = cp.tile([1, 1], f32, name="u2")
            nc.vector.tensor_sub(u2, a2v, b2v)
            isg2 = _rsqrt_newton(nc, cp, u2, "hd", iters=1)

            # broadcast m2, isg2 across partitions via rank-1 PE matmul
            mi2 = cp.tile([1, 2], f32, name="mi2")
            nc.vector.tensor_copy(mi2[:, 0:1], m2)
            nc.vector.tensor_copy(mi2[:, 1:2], isg2)
            mi2b_ps = pp.tile([128, 2], f32, name="mi2b_ps", tag="psE")
            nc.tensor.matmul(mi2b_ps, ones_row, mi2, start=True, stop=True)
            mi2b = cp.tile([128, 2], f32, name="mi2b")
            nc.vector.tensor_copy(mi2b, mi2b_ps)
            xn2 = cp.tile([128, 1], f32, name="xn2")
            nc.vector.tensor_scalar(xn2, st_in[:, 0:1], mi2b[:, 0:1],
                                    mi2b[:, 1:2], ALU.subtract, ALU.mult)
            relu2 = cp.tile([128, 1], f32, name="relu2")
            nc.scalar.activation(relu2, xn2, AF.Relu, bias=ln2_b, scale=ln2_g)

            out_ps = pp.tile([1, 1], f32, name="out_ps", tag="psD")
            nc.tensor.matmul(out_ps, relu2, fc2_col, start=True, stop=True)
            out_sb = cp.tile([1, 1], f32, name="out_sb")
            nc.vector.tensor_add(out_sb, out_ps, fc2_b)
            nc.sync.dma_start(out_d.ap(), out_sb)

    _split_excess_waits(nc)
    return nc


_PROGRAM = None


def _get_program():
    global _PROGRAM
    if _PROGRAM is None:
        _PROGRAM = build_program()
    return _PROGRAM


def make_in_maps(inputs):
    """Shard FULL inputs into per-core in_maps (host-side: views/pads only)."""
    def pad_shard(a):
        a = np.asarray(a, dtype=np.int64)
        p = np.full(E_PAD, 32, dtype=np.int64)
        p[:E_FULL] = a
        return [np.ascontiguousarray(p[c * EPC:(c + 1) * EPC])
                .view(np.int16).reshape(128, 4 * W) for c in range(NCORES)]

    es_s = pad_shard(inputs["es"])
    ed_s = pad_shard(inputs["ed"])
    ef_s = pad_shard(inputs["ef"])

    def f(x, shape):
        return np.ascontiguousarray(
            np.asarray(x, dtype=np.float32).reshape(shape))

    common = {
        "nt": np.ascontiguousarray(np.asarray(inputs["nt"], np.int64))
        .view(np.int16).reshape(32, 4),
        "tr": np.ascontiguousarray(np.asarray(inputs["tr"], np.int64))
        .view(np.int16).reshape(32, 4),
        "ne_w": f(inputs["ne_w"], (20, DIM)),
        "te_w": f(inputs["te_w"], (6, DIM)),
        "ef_w": f(inputs["ef_w"], (6, DIM)),
        "w_ih": f(inputs["w_ih"], (384, DIM)),
        "w_hh": f(inputs["w_hh"], (384, DIM)),
        "b_ih": f(inputs["b_ih"], (1, 384)),
        "b_hh": f(inputs["b_hh"], (1, 384)),
        "ln_g": f(inputs["ln_g"], (DIM, 1)),
        "ln_b": f(inputs["ln_b"], (DIM, 1)),
        "fc1_w": f(inputs["fc1_w"], (DIM, 2 * DIM)),
        "fc1_b": f(inputs["fc1_b"], (DIM, 1)),
        "ln2_g": f(inputs["ln2_g"], (DIM, 1)),
        "ln2_b": f(inputs["ln2_b"], (DIM, 1)),
        "fc2_w": f(inputs["fc2_w"], (1, DIM)),
        "fc2_b": f(inputs["fc2_b"], (1, 1)),
        "ident128": np.eye(128, dtype=np.float32),
        "ones_row": np.ones((1, 128), np.float32),
        "ones_col": np.ones((128, 1), np.float32),
        "iota_mat": np.broadcast_to(
            np.arange(32, dtype=np.float32), (32, 32)).copy(),
        "iota_row_bf": np.broadcast_to(
            np.arange(32, dtype=np.float32).astype(ml_dtypes.bfloat16),
            (128, 32)).copy(),
    }
    in_maps = []
    for c in range(NCORES):
        m = dict(common)
        m["es"] = es_s[c]
        m["ed"] = ed_s[c]
        m["ef"] = ef_s[c]
        in_maps.append(m)
    return in_maps


def kernel(**inputs) -> np.ndarray:
    nc = _get_program()
    in_maps = make_in_maps(inputs)
    res = run_bass_kernel_spmd(nc, in_maps, core_ids=list(range(NCORES)))
    return np.asarray(res.results[0]["out"], np.float32).reshape(())
